# revision 34
# baseline (speedup 1.0000x reference)
"""Trainium2 Bass kernel for BasicInteractionNetworkModule.

Data-parallel over batch (B=16) across 8 NeuronCores, 2 batches/core.

Math (per batch b):
  senders   = S^T @ O          [R, 128]   (S = sender_relations [128, R])
  receivers = R_rel^T @ O      [R, 128]
  rel_x = [senders, receivers, info]   [R, 320]
  h = relu-MLP(rel_x): 320 -> 256 -> 256 -> 256 -> 128 (relu after every layer)
  eff_recv = R_rel @ effects   [128, 128]
  obj_x = [O, ext, eff_recv]   [128, 288]
  out = relu-MLP2(obj_x): 288 -> 256 -> 256 -> 128 (no final relu)

Device strategy (v7): feature-major relation MLP with stationary weights;
layer-1 folds the sender/receiver projections via A_s = O @ rw1[:128],
A_r = O @ rw1[128:256] so S/R stream from DRAM as moving operands.  The
K=64 info contribution of layer 1 runs as a PE row-tiled pair: both hid
halves' info matmuls occupy array rows 0:63 / 64:127 concurrently
(moving = replicated info rows at SBUF partitions 0:64 / 64:128), so the
info pass costs ~1 column-stream instead of 2.  The two per-core batches
run as ONE continuous 36-slot software pipeline (slot s emits L1(s),
L3(s-1), L2(s) as N=512 "host" matmuls); the N=128 matmuls of L4
(relation-major effects, stationary = H3 slices) and of the
relation->object aggregation (one PSUM bank accumulated across a batch's
127 slices) are enqueued as "small" thunks and pumped one-per-host from
a FIFO, so every LDWEIGHTS hides under a 512-col stream and the PE never
sees a run of small MMs.  The object MLP is enqueued the same way right
after its batch's last aggregation slice, so the batch boundary has no
pipeline drain.  Producer->consumer emission distance is >= one section
everywhere (L1 ACT-evacs are consumed by L2 only after the L3 section).
Evacuations: L1/L2 on ACT (bias+relu), L3 on DVE, L4's free-dim bias via
DVE tensor_tensor + relu (ACT for the tail chunks; obj-MLP evacs split
ACT/DVE per half so the tail chain halves retire in parallel).  DMA:
chunk streams on the sync HWDGE queue in consumption order (ramp RTs
deferred behind S/R, ramp I-chunks pre-issued on SWDGE, only 3 small
weights on the scalar/ACT queue so evacuations are never queued behind
DMA triggers — the HWDGE trigger costs ~0.65us of issuing-engine time).
PSUM: 5 rotating host banks + 2 L4 banks + 1 aggregation bank.  A few
dependency-free warm-up matmuls ramp the PE HAM clock gate (4/8 ->
8/8 after ~3.4us of activity) while the first DMAs land.
"""

import numpy as np
import ml_dtypes
from collections import deque

B, N_OBJ, N_REL = 16, 128, 16256
OBJ_D, REL_D, EFF_D, EXT_D, OUT_D = 128, 64, 128, 32, 128
HID = 256
N_CORES = 8
B_CORE = B // N_CORES  # 2
M_CHUNK = 1024
N_WARM = 4

_CACHE = {}


def _chunks():
    out = [(0, 512), (512, 512)]
    base = 1024
    while base < N_REL:
        mc = min(M_CHUNK, N_REL - base)
        if mc == 896:
            out.append((base, 512))
            out.append((base + 512, 384))
            base += 896
        else:
            out.append((base, mc))
            base += mc
    return out


def _mtiles(mc):
    out = []
    base = 0
    while base < mc:
        n = min(512, mc - base)
        out.append((base, n))
        base += n
    return out


def build_kernel():
    from concourse import bacc
    import concourse.mybir as mybir
    import concourse.tile as tile

    F32 = mybir.dt.float32
    BF16 = mybir.dt.bfloat16
    RELU = mybir.ActivationFunctionType.Relu
    ADD = mybir.AluOpType.add
    MAX = mybir.AluOpType.max

    nc = bacc.Bacc(None)

    # per-core inputs
    S_d = nc.dram_tensor("s_rel", [B_CORE, N_OBJ, N_REL], BF16, kind="ExternalInput")
    R_d = nc.dram_tensor("r_rel", [B_CORE, N_OBJ, N_REL], BF16, kind="ExternalInput")
    IT_d = nc.dram_tensor("info_t", [B_CORE, 128, N_REL], BF16, kind="ExternalInput")
    RT_d = nc.dram_tensor("r_rel_t", [B_CORE, N_REL, N_OBJ], BF16, kind="ExternalInput")
    OT_d = nc.dram_tensor("objs_t", [B_CORE, OBJ_D, N_OBJ], BF16, kind="ExternalInput")
    XT_d = nc.dram_tensor("ext_t", [B_CORE, 128, N_OBJ], BF16, kind="ExternalInput")

    rw1s_d = nc.dram_tensor("rw1s", [128, HID], BF16, kind="ExternalInput")
    rw1r_d = nc.dram_tensor("rw1r", [128, HID], BF16, kind="ExternalInput")
    rw1i_d = nc.dram_tensor("rw1i", [128, 128], BF16, kind="ExternalInput")
    rw2_d = nc.dram_tensor("rw2f", [128, 2, HID], BF16, kind="ExternalInput")
    rw3_d = nc.dram_tensor("rw3f", [128, 2, HID], BF16, kind="ExternalInput")
    rw4_d = nc.dram_tensor("rw4b", [128, 2, EFF_D], BF16, kind="ExternalInput")
    ow1o_d = nc.dram_tensor("ow1o", [128, HID], BF16, kind="ExternalInput")
    ow1x_d = nc.dram_tensor("ow1x", [128, HID], BF16, kind="ExternalInput")
    ow1e_d = nc.dram_tensor("ow1e", [128, HID], BF16, kind="ExternalInput")
    ow2_d = nc.dram_tensor("ow2f", [128, 2, HID], BF16, kind="ExternalInput")
    ow3_d = nc.dram_tensor("ow3f", [128, 2, OUT_D], BF16, kind="ExternalInput")

    rb1_d = nc.dram_tensor("rb1c", [128, 2], F32, kind="ExternalInput")
    rb2_d = nc.dram_tensor("rb2c", [128, 2], F32, kind="ExternalInput")
    rb3_d = nc.dram_tensor("rb3c", [128, 2], F32, kind="ExternalInput")
    ob1_d = nc.dram_tensor("ob1c", [128, 2], F32, kind="ExternalInput")
    ob2_d = nc.dram_tensor("ob2c", [128, 2], F32, kind="ExternalInput")
    ob3_d = nc.dram_tensor("ob3r", [128, OUT_D], F32, kind="ExternalInput")
    rb4bc_d = nc.dram_tensor("rb4bc", [128, 512], BF16, kind="ExternalInput")

    out_d = nc.dram_tensor("out", [B_CORE, N_OBJ, OUT_D], F32, kind="ExternalOutput")

    chs = _chunks()
    LAST_CI = len(chs) - 1
    slots = [(b, ci) for b in range(B_CORE) for ci in range(len(chs))]
    NS = len(slots)

    with tile.TileContext(nc) as tc:
        with (
            tc.tile_pool(name="wts", bufs=1) as wts,
            tc.tile_pool(name="perb", bufs=1) as perb,
            tc.tile_pool(name="cin", bufs=5) as cin,
            tc.tile_pool(name="acts", bufs=3) as acts,
            tc.tile_pool(name="psp", bufs=5, space="PSUM") as psp,
            tc.tile_pool(name="ps4", bufs=2, space="PSUM") as ps4p,
            tc.tile_pool(name="psa", bufs=1, space="PSUM") as psap,
        ):
            # ---- small-matmul FIFO: one small per N=512 host matmul ----
            pend = deque()

            def pump():
                if pend:
                    pend.popleft()()

            def hostmm(*a, **k):
                nc.tensor.matmul(*a, skip_group_check=True, **k)
                pump()

            # ---- warm-up: PE busy early so the HAM clock gate goes 8/8
            # while the first DMAs land ----
            # warm operand memset rides gpsimd, whose preamble ends earliest
            warm = wts.tile([128, 512], BF16)
            nc.gpsimd.memset(warm, 0.0)
            psW = psp.tile([128, 512], F32, tag="ps")
            for _ in range(N_WARM):
                nc.tensor.matmul(psW, warm[:, :128], warm, start=True, stop=True)

            # ---- setup DMAs.  sync: what the very first hosts need, then
            # the per-chunk streams (emitted in the slot loop).  scalar:
            # second-wave setup.  gpsimd: bulk weights. ----
            rw1s = wts.tile([128, HID], BF16)
            rw1r = wts.tile([128, HID], BF16)
            rw1i = wts.tile([128, 128], BF16)
            rb1 = wts.tile([128, 2], F32)
            OTs = [perb.tile([OBJ_D, N_OBJ], BF16, tag=f"OT{b}", name=f"OT{b}")
                   for b in range(B_CORE)]
            XTs = [perb.tile([128, N_OBJ], BF16, tag=f"XT{b}", name=f"XT{b}")
                   for b in range(B_CORE)]
            nc.sync.dma_start(rw1s, rw1s_d[:])
            nc.sync.dma_start(OTs[0], OT_d[0])
            nc.scalar.dma_start(rw1r, rw1r_d[:])
            nc.scalar.dma_start(rw1i, rw1i_d[:])
            # rb1 is read by L1 evacs from slot 0 on — its DMA must be
            # emitted before those reads or Tile orders it as a WAR
            nc.scalar.dma_start(rb1, rb1_d[:])

            # I-chunks for the ramp ride SWDGE ahead of the bulk weights so
            # the scalar (ACT) queue stays free for L1 evacuations
            I_ramp = []
            for ci in range(3):
                base, mc = chs[ci]
                I_c = cin.tile([128, M_CHUNK], BF16, tag="I_c", name=f"I_r{ci}")
                nc.gpsimd.dma_start(I_c[:, :mc], IT_d[0, :, base:base + mc])
                I_ramp.append(I_c)

            rw2 = wts.tile([128, 2, HID], BF16)
            rw3 = wts.tile([128, 2, HID], BF16)
            rw4 = wts.tile([128, 2, EFF_D], BF16)
            rb4bc = wts.tile([128, 512], BF16)
            ow1o = wts.tile([128, HID], BF16)
            ow1x = wts.tile([128, HID], BF16)
            ow1e = wts.tile([128, HID], BF16)
            ow2 = wts.tile([128, 2, HID], BF16)
            ow3 = wts.tile([128, 2, OUT_D], BF16)
            rb2 = wts.tile([128, 2], F32)
            rb3 = wts.tile([128, 2], F32)
            ob1 = wts.tile([128, 2], F32)
            ob2 = wts.tile([128, 2], F32)
            ob3 = wts.tile([128, OUT_D], F32)
            for t, dsrc in [(rw2, rw2_d), (rb2, rb2_d), (rw3, rw3_d),
                            (rb3, rb3_d), (rw4, rw4_d), (rb4bc, rb4bc_d)]:
                nc.gpsimd.dma_start(t, dsrc[:])
            obj_wt_dmas = [(ow1o, ow1o_d), (ow1x, ow1x_d), (ow1e, ow1e_d),
                           (ow2, ow2_d), (ow3, ow3_d),
                           (ob1, ob1_d), (ob2, ob2_d), (ob3, ob3_d)]

            # ---- per-slot / per-batch state ----
            rt_defer = []
            ST, H1s, H2s, H3s, E3s = {}, {}, {}, {}, {}
            AsAr, psagg, aggfirst = {}, {}, {}

            def emit_AsAr(b):
                As = perb.tile([N_OBJ, HID], BF16, tag=f"As{b}")
                Ar = perb.tile([N_OBJ, HID], BF16, tag=f"Ar{b}")
                psA = psp.tile([128, 512], F32, tag="ps")
                hostmm(psA[:, :HID], OTs[b], rw1s, start=True, stop=True)
                nc.vector.tensor_copy(As, psA[:, :HID])
                psA2 = psp.tile([128, 512], F32, tag="ps")
                hostmm(psA2[:, :HID], OTs[b], rw1r, start=True, stop=True)
                nc.vector.tensor_copy(Ar, psA2[:, :HID])
                AsAr[b] = (As, Ar)

            def chunk_dmas(s):
                b, ci = slots[s]
                base, mc = chs[ci]
                S_c = cin.tile([N_OBJ, M_CHUNK], BF16, tag="S_c")
                R_c = cin.tile([N_OBJ, M_CHUNK], BF16, tag="R_c")
                I_c = (I_ramp[s] if s < 3 else
                       cin.tile([128, M_CHUNK], BF16, tag="I_c"))
                RT_c = cin.tile([128, M_CHUNK // 128, N_OBJ], BF16, tag="RT_c")
                ns = mc // 128
                rt_src = RT_d[b, base:base + mc, :].rearrange(
                    "(s p) o -> p s o", p=128)
                if s < 3:
                    # ramp: S/R on sync, I pre-issued on SWDGE (I_ramp), and
                    # all three RTs (not needed until the aggregation) deferred
                    # behind S2/R2 so the ~0.65us/trigger HWDGE rate feeds the
                    # PE in consumption order and HAM never re-throttles
                    nc.sync.dma_start(S_c[:, :mc], S_d[b, :, base:base + mc])
                    nc.sync.dma_start(R_c[:, :mc], R_d[b, :, base:base + mc])
                    rt_defer.append((RT_c, ns, rt_src))
                    if s == 2:
                        for RT_p, ns_p, src_p in rt_defer:
                            nc.sync.dma_start(RT_p[:, :ns_p, :], src_p)
                        rt_defer.clear()
                else:
                    nc.sync.dma_start(S_c[:, :mc], S_d[b, :, base:base + mc])
                    nc.sync.dma_start(R_c[:, :mc], R_d[b, :, base:base + mc])
                    nc.sync.dma_start(I_c[:, :mc], IT_d[b, :, base:base + mc])
                    nc.sync.dma_start(RT_c[:, :ns, :], rt_src)
                ST[s] = (S_c, R_c, I_c, RT_c)

            def emit_L1(s):
                b, ci = slots[s]
                base, mc = chs[ci]
                As, Ar = AsAr[b]
                S_c, R_c, I_c, _ = ST[s]
                H1 = acts.tile([128, 2, M_CHUNK], BF16, tag="H1")
                for mt, n in _mtiles(mc):
                    sl = slice(mt, mt + n)
                    ps0 = psp.tile([128, 512], F32, tag="ps", name="ps0")
                    ps1 = psp.tile([128, 512], F32, tag="ps", name="ps1")
                    hostmm(ps0[:, :n], As[:, 0:128], S_c[:, sl], start=True, stop=False)
                    hostmm(ps0[:, :n], Ar[:, 0:128], R_c[:, sl], start=False, stop=False)
                    hostmm(ps1[:, :n], As[:, 128:256], S_c[:, sl], start=True, stop=False)
                    hostmm(ps1[:, :n], Ar[:, 128:256], R_c[:, sl], start=False, stop=False)
                    # adjacent K=64 row-tiled pair: rows 0:63 and 64:127 of
                    # the PE array compute both hid halves concurrently
                    nc.tensor.matmul(ps0[:, :n], rw1i[0:64, :], I_c[0:64, sl],
                                     start=False, stop=True, skip_group_check=True)
                    nc.tensor.matmul(ps1[:, :n], rw1i[64:128, :], I_c[64:128, sl],
                                     start=False, stop=True, skip_group_check=True)
                    pump()
                    nc.scalar.activation(H1[:, 0, sl], ps0[:, :n], RELU,
                                         bias=rb1[:, 0:1], scale=1.0)
                    nc.scalar.activation(H1[:, 1, sl], ps1[:, :n], RELU,
                                         bias=rb1[:, 1:2], scale=1.0)
                H1s[s] = H1

            def emit_L2(s):
                b, ci = slots[s]
                base, mc = chs[ci]
                H1 = H1s.pop(s)
                H2 = acts.tile([128, 2, M_CHUNK], BF16, tag="H2")
                for p2 in range(2):
                    h = slice(p2 * 128, (p2 + 1) * 128)
                    for mt, n in _mtiles(mc):
                        sl = slice(mt, mt + n)
                        ps = psp.tile([128, 512], F32, tag="ps")
                        hostmm(ps[:, :n], rw2[:, 0, h], H1[:, 0, sl], start=True, stop=False)
                        hostmm(ps[:, :n], rw2[:, 1, h], H1[:, 1, sl], start=False, stop=True)
                        nc.scalar.activation(H2[:, p2, sl], ps[:, :n], RELU,
                                             bias=rb2[:, p2:p2 + 1], scale=1.0)
                H2s[s] = H2

            def emit_L3(s):
                b, ci = slots[s]
                base, mc = chs[ci]
                H2 = H2s.pop(s)
                H3 = acts.tile([128, 2, M_CHUNK], BF16, tag="H3")
                for p2 in range(2):
                    h = slice(p2 * 128, (p2 + 1) * 128)
                    for mt, n in _mtiles(mc):
                        sl = slice(mt, mt + n)
                        ps = psp.tile([128, 512], F32, tag="ps")
                        hostmm(ps[:, :n], rw3[:, 0, h], H2[:, 0, sl], start=True, stop=False)
                        hostmm(ps[:, :n], rw3[:, 1, h], H2[:, 1, sl], start=False, stop=True)
                        if ci >= 16:
                            # small tail chunks: ACT is underloaded, DVE is
                            # the drain bottleneck
                            nc.scalar.activation(H3[:, p2, sl], ps[:, :n], RELU,
                                                 bias=rb3[:, p2:p2 + 1], scale=1.0)
                        else:
                            nc.vector.tensor_scalar(H3[:, p2, sl], ps[:, :n],
                                                    rb3[:, p2:p2 + 1], 0.0, ADD, MAX)
                H3s[s] = H3

            def enqueue_L4(s):
                b, ci = slots[s]
                base, mc = chs[ci]
                H3 = H3s.pop(s)
                ns = mc // 128
                E3 = acts.tile([128, M_CHUNK], BF16, tag="E3")
                E3s[s] = E3
                for g in range(0, ns, 4):
                    ge = min(g + 4, ns)
                    span = (ge - g) * 128
                    gsl = slice(g * 128, g * 128 + span)
                    ps4 = ps4p.tile([128, 512], F32, tag="ps4")
                    Etmp = acts.tile([128, 512], BF16, tag="Etmp")
                    for k in range(2):
                        for sj in range(g, ge):
                            last = (k == 1 and sj == ge - 1)

                            def t(k=k, sj=sj, g=g, ge=ge, ps4=ps4, Etmp=Etmp,
                                  span=span, gsl=gsl, last=last, H3=H3, E3=E3):
                                sl = slice(sj * 128, (sj + 1) * 128)
                                psl = slice((sj - g) * 128, (sj - g + 1) * 128)
                                nc.tensor.matmul(ps4[:, psl], H3[:, k, sl], rw4[:, k, :],
                                                 start=(k == 0 and sj == g),
                                                 stop=last,
                                                 skip_group_check=True)
                                if last:
                                    nc.vector.tensor_tensor(Etmp[:, :span], ps4[:, :span],
                                                            rb4bc[:, :span], ADD)
                                    if ci >= 16:
                                        nc.scalar.activation(E3[:, gsl], Etmp[:, :span],
                                                             RELU, bias=0.0, scale=1.0)
                                    else:
                                        nc.vector.tensor_scalar(E3[:, gsl], Etmp[:, :span],
                                                                0.0, None, MAX)
                            pend.append(t)

            def enqueue_agg(s):
                b, ci = slots[s]
                base, mc = chs[ci]
                ns = mc // 128
                E3 = E3s.pop(s)
                RT_c = ST.pop(s)[3]
                if b not in psagg:
                    psagg[b] = psap.tile([128, 512], F32, tag="agg",
                                         name=f"psagg{b}")
                    aggfirst[b] = [True]
                pa, first = psagg[b], aggfirst[b]
                last_chunk = (ci == LAST_CI)
                for sj in range(ns):
                    def t(sj=sj, pa=pa, first=first, E3=E3, RT_c=RT_c,
                          stop=(last_chunk and sj == ns - 1)):
                        nc.tensor.matmul(pa[:, :N_OBJ], E3[:, sj * 128:(sj + 1) * 128],
                                         RT_c[:, sj, :], start=first[0], stop=stop,
                                         skip_group_check=True)
                        first[0] = False
                    pend.append(t)

            def enqueue_obj(b):
                pa = psagg[b]
                effT = perb.tile([EFF_D, N_OBJ], BF16, tag=f"effT{b}")
                G1 = perb.tile([128, 2, N_OBJ], BF16, tag=f"G1{b}")
                G2 = perb.tile([128, 2, N_OBJ], BF16, tag=f"G2{b}")
                ob = perb.tile([N_OBJ, OUT_D], F32, tag=f"ob{b}")
                pend.append(lambda: nc.vector.tensor_copy(effT, pa[:, :N_OBJ]))
                for p2 in range(2):
                    ps = psp.tile([128, 512], F32, tag="ps")
                    h = slice(p2 * 128, (p2 + 1) * 128)
                    for wi, (wt, mv) in enumerate([(ow1o, OTs[b]), (ow1x, XTs[b]),
                                                   (ow1e, effT)]):
                        def t(ps=ps, h=h, wt=wt, mv=mv, wi=wi, p2=p2):
                            nc.tensor.matmul(ps[:, :N_OBJ], wt[:, h], mv,
                                             start=(wi == 0), stop=(wi == 2),
                                             skip_group_check=True)
                            if wi == 2:
                                # split evacs across ACT/DVE so the two
                                # halves retire in parallel on the tail path
                                if p2 == 0:
                                    nc.scalar.activation(G1[:, p2, :], ps[:, :N_OBJ],
                                                         RELU, bias=ob1[:, p2:p2 + 1],
                                                         scale=1.0)
                                else:
                                    nc.vector.tensor_scalar(G1[:, p2, :], ps[:, :N_OBJ],
                                                            ob1[:, p2:p2 + 1], 0.0,
                                                            ADD, MAX)
                        pend.append(t)
                for p2 in range(2):
                    ps = psp.tile([128, 512], F32, tag="ps")
                    h = slice(p2 * 128, (p2 + 1) * 128)
                    for k in range(2):
                        def t(ps=ps, h=h, k=k, p2=p2):
                            nc.tensor.matmul(ps[:, :N_OBJ], ow2[:, k, h], G1[:, k, :],
                                             start=(k == 0), stop=(k == 1),
                                             skip_group_check=True)
                            if k == 1:
                                if p2 == 0:
                                    nc.scalar.activation(G2[:, p2, :], ps[:, :N_OBJ],
                                                         RELU, bias=ob2[:, p2:p2 + 1],
                                                         scale=1.0)
                                else:
                                    nc.vector.tensor_scalar(G2[:, p2, :], ps[:, :N_OBJ],
                                                            ob2[:, p2:p2 + 1], 0.0,
                                                            ADD, MAX)
                        pend.append(t)
                pso = psp.tile([128, 512], F32, tag="ps")
                for k in range(2):
                    def t(pso=pso, k=k, b=b, ob=ob):
                        nc.tensor.matmul(pso[:, :OUT_D], G2[:, k, :], ow3[:, k, :],
                                         start=(k == 0), stop=(k == 1),
                                         skip_group_check=True)
                        if k == 1:
                            if b == 0:
                                nc.vector.tensor_tensor(ob, pso[:, :OUT_D], ob3, ADD)
                                nc.gpsimd.dma_start(out_d[b], ob)
                            else:
                                # final batch: halves so the first DMA
                                # overlaps the second bias-add on the tail
                                nc.vector.tensor_tensor(ob[:, 0:64], pso[:, 0:64],
                                                        ob3[:, 0:64], ADD)
                                nc.sync.dma_start(out_d[b, :, 0:64], ob[:, 0:64])
                                nc.vector.tensor_tensor(ob[:, 64:128], pso[:, 64:128],
                                                        ob3[:, 64:128], ADD)
                                nc.sync.dma_start(out_d[b, :, 64:128], ob[:, 64:128])
                    pend.append(t)

            # ---- the continuous pipeline over both batches ----
            emit_AsAr(0)
            for s in range(NS):
                chunk_dmas(s)
                if s == 2 and obj_wt_dmas:
                    for t, dsrc in obj_wt_dmas:
                        nc.gpsimd.dma_start(t, dsrc[:])
                    obj_wt_dmas.clear()
                if s == 3:
                    nc.gpsimd.dma_start(OTs[1], OT_d[1])
                    nc.gpsimd.dma_start(XTs[0], XT_d[0])
                    nc.gpsimd.dma_start(XTs[1], XT_d[1])
                if s == 16:
                    emit_AsAr(1)
                if s >= 2:
                    q = s - 2
                    enqueue_agg(q)
                    if slots[q][1] == LAST_CI:
                        enqueue_obj(slots[q][0])
                emit_L1(s)
                if s >= 1:
                    emit_L3(s - 1)
                    enqueue_L4(s - 1)
                emit_L2(s)
            # drain
            emit_L3(NS - 1)
            enqueue_L4(NS - 1)
            enqueue_agg(NS - 2)
            enqueue_agg(NS - 1)
            enqueue_obj(slots[NS - 1][0])
            while pend:
                pump()

    nc.compile()
    return nc


def _prep_inputs(objects, sender_relations, receiver_relations, relation_info,
                 external_effect_info, rw1, rb1, rw2, rb2, rw3, rb3, rw4, rb4,
                 ow1, ob1, ow2, ob2, ow3, ob3):
    bf16 = ml_dtypes.bfloat16
    f32 = np.float32

    def a(x):
        return np.ascontiguousarray(np.asarray(x, dtype=f32))

    objects = a(objects); sender_relations = a(sender_relations)
    receiver_relations = a(receiver_relations); relation_info = a(relation_info)
    external_effect_info = a(external_effect_info)
    rw1, rb1, rw2, rb2, rw3, rb3, rw4, rb4 = map(a, (rw1, rb1, rw2, rb2, rw3, rb3, rw4, rb4))
    ow1, ob1, ow2, ob2, ow3, ob3 = map(a, (ow1, ob1, ow2, ob2, ow3, ob3))

    # relation info, transposed; rows 64:128 replicate rows 0:64 so the two
    # PE-row-tiled K=64 info matmuls can stream from partitions 0:64 / 64:128
    info_t = np.zeros((B, 128, N_REL), dtype=bf16)
    info_t[:, :REL_D, :] = relation_info.transpose(0, 2, 1).astype(bf16)
    info_t[:, REL_D:2 * REL_D, :] = info_t[:, :REL_D, :]
    s_bf = sender_relations.astype(bf16)
    r_bf = receiver_relations.astype(bf16)
    r_rel_t = np.ascontiguousarray(
        receiver_relations.transpose(0, 2, 1)).astype(bf16)
    objs_t = np.ascontiguousarray(objects.transpose(0, 2, 1)).astype(bf16)
    # ext, transposed and K-padded 32 -> 128
    ext_t = np.zeros((B, 128, N_OBJ), dtype=bf16)
    ext_t[:, :EXT_D, :] = external_effect_info.transpose(0, 2, 1).astype(bf16)

    # rw1i packed for row-tiling: rows 0:64 = info weights for hid half 0,
    # rows 64:128 = info weights for hid half 1
    rw1i_pad = np.zeros((128, 128), dtype=bf16)
    rw1i_pad[:REL_D] = rw1[256:320, 0:128].astype(bf16)
    rw1i_pad[64:64 + REL_D] = rw1[256:320, 128:256].astype(bf16)
    ow1x_pad = np.zeros((128, HID), dtype=bf16)
    ow1x_pad[:EXT_D] = ow1[128:160].astype(bf16)
    rb4bc = np.ascontiguousarray(
        np.broadcast_to(np.tile(rb4, 4).astype(bf16)[None, :], (128, 512)))

    shared = {
        "rw1s": rw1[0:128].astype(bf16),
        "rw1r": rw1[128:256].astype(bf16),
        "rw1i": rw1i_pad,
        "rw2f": np.ascontiguousarray(rw2.reshape(2, 128, HID).transpose(1, 0, 2)).astype(bf16),
        "rw3f": np.ascontiguousarray(rw3.reshape(2, 128, HID).transpose(1, 0, 2)).astype(bf16),
        "rw4b": np.ascontiguousarray(rw4.reshape(2, 128, EFF_D).transpose(1, 0, 2)).astype(bf16),
        "ow1o": ow1[0:128].astype(bf16),
        "ow1x": ow1x_pad,
        "ow1e": ow1[160:288].astype(bf16),
        "ow2f": np.ascontiguousarray(ow2.reshape(2, 128, HID).transpose(1, 0, 2)).astype(bf16),
        "ow3f": np.ascontiguousarray(ow3.reshape(2, 128, OUT_D).transpose(1, 0, 2)).astype(bf16),
        "rb1c": np.ascontiguousarray(rb1.reshape(2, 128).T),
        "rb2c": np.ascontiguousarray(rb2.reshape(2, 128).T),
        "rb3c": np.ascontiguousarray(rb3.reshape(2, 128).T),
        "ob1c": np.ascontiguousarray(ob1.reshape(2, 128).T),
        "ob2c": np.ascontiguousarray(ob2.reshape(2, 128).T),
        "ob3r": np.ascontiguousarray(np.broadcast_to(ob3[None, :], (128, OUT_D))),
        "rb4bc": rb4bc,
    }

    in_maps = []
    for c in range(N_CORES):
        sl = slice(c * B_CORE, (c + 1) * B_CORE)
        m = dict(shared)
        m["s_rel"] = s_bf[sl]
        m["r_rel"] = r_bf[sl]
        m["info_t"] = np.ascontiguousarray(info_t[sl])
        m["r_rel_t"] = r_rel_t[sl]
        m["objs_t"] = objs_t[sl]
        m["ext_t"] = np.ascontiguousarray(ext_t[sl])
        in_maps.append(m)
    return in_maps


def run(in_maps, **spmd_kwargs):
    from concourse.bass_utils import run_bass_kernel_spmd

    if "nc" not in _CACHE:
        _CACHE["nc"] = build_kernel()
    return run_bass_kernel_spmd(_CACHE["nc"], in_maps,
                                core_ids=list(range(N_CORES)), **spmd_kwargs)


def kernel(**inputs) -> np.ndarray:
    in_maps = _prep_inputs(**inputs)
    res = run(in_maps)
    out = np.concatenate([r["out"].reshape(-1, OUT_D) for r in res.results], axis=0)
    return np.ascontiguousarray(out, dtype=np.float32)


# revision 35
# speedup vs baseline: 1.2029x; 1.2029x over previous
"""Trainium2 Bass kernel for BasicInteractionNetworkModule.

Data-parallel over batch (B=16) across 8 NeuronCores, 2 batches/core.

Math (per batch b):
  senders   = S^T @ O          [R, 128]   (S = sender_relations [128, R])
  receivers = R_rel^T @ O      [R, 128]
  rel_x = [senders, receivers, info]   [R, 320]
  h = relu-MLP(rel_x): 320 -> 256 -> 256 -> 256 -> 128 (relu after every layer)
  eff_recv = R_rel @ effects   [128, 128]
  obj_x = [O, ext, eff_recv]   [128, 288]
  out = relu-MLP2(obj_x): 288 -> 256 -> 256 -> 128 (no final relu)

Device strategy (v6): feature-major relation MLP with stationary weights;
layer-1 folds the sender/receiver projections via A_s = O @ rw1[:128],
A_r = O @ rw1[128:256] so S/R stream from DRAM as moving operands; all
small-K matmuls are zero-padded to K=128 (PE time depends only on moving
columns, padding costs nothing on the PE).  The two per-core batches run
as ONE continuous 36-slot software pipeline (slot s emits L1(s), L3(s-1),
L2(s) as N=512 "host" matmuls); the N=128 matmuls of L4 (relation-major
effects, stationary = H3 slices) and of the relation->object aggregation
(one PSUM bank accumulated across a batch's 127 slices) are enqueued as
"small" thunks and pumped one-per-host from a FIFO, so every LDWEIGHTS
hides under a 512-col stream and the PE never sees a run of small MMs.
The object MLP is enqueued the same way right after its batch's last
aggregation slice, so the batch boundary has no pipeline drain.
Producer->consumer emission distance is >= one section everywhere
(L1 ACT-evacs are consumed by L2 only after the L3 section) which kills
the per-macro-iteration PE stalls on evacuation semaphores seen in v5.
Evacuations: L1/L2 on ACT (bias+relu), L3 on DVE, L4's free-dim bias via
DVE tensor_tensor + DVE relu.  Object MLP runs in bf16.  PSUM: 7
rotating transient banks + 1 aggregation bank.  Warm-up matmuls (gpsimd
memset, ~8) ramp the PE HAM clock gate while the first DMAs land.
"""

import numpy as np
import ml_dtypes
from collections import deque

B, N_OBJ, N_REL = 16, 128, 16256
OBJ_D, REL_D, EFF_D, EXT_D, OUT_D = 128, 64, 128, 32, 128
HID = 256
N_CORES = 8
B_CORE = B // N_CORES  # 2
M_CHUNK = 1024
N_WARM = 12

_CACHE = {}


def _chunks():
    out = [(0, 512), (512, 512)]
    base = 1024
    while base < N_REL:
        mc = min(M_CHUNK, N_REL - base)
        if mc == 896:
            out.append((base, 512))
            out.append((base + 512, 384))
            base += 896
        else:
            out.append((base, mc))
            base += mc
    return out


def _mtiles(mc):
    out = []
    base = 0
    while base < mc:
        n = min(512, mc - base)
        out.append((base, n))
        base += n
    return out


def build_kernel():
    from concourse import bacc
    import concourse.mybir as mybir
    import concourse.tile as tile

    F32 = mybir.dt.float32
    BF16 = mybir.dt.bfloat16
    RELU = mybir.ActivationFunctionType.Relu
    ADD = mybir.AluOpType.add
    MAX = mybir.AluOpType.max

    nc = bacc.Bacc(None)

    # per-core inputs
    S_d = nc.dram_tensor("s_rel", [B_CORE, N_OBJ, N_REL], BF16, kind="ExternalInput")
    R_d = nc.dram_tensor("r_rel", [B_CORE, N_OBJ, N_REL], BF16, kind="ExternalInput")
    IT_d = nc.dram_tensor("info_t", [B_CORE, 128, N_REL], BF16, kind="ExternalInput")
    RT_d = nc.dram_tensor("r_rel_t", [B_CORE, N_REL, N_OBJ], BF16, kind="ExternalInput")
    OT_d = nc.dram_tensor("objs_t", [B_CORE, OBJ_D, N_OBJ], BF16, kind="ExternalInput")
    XT_d = nc.dram_tensor("ext_t", [B_CORE, 128, N_OBJ], BF16, kind="ExternalInput")

    rw1s_d = nc.dram_tensor("rw1s", [128, HID], BF16, kind="ExternalInput")
    rw1r_d = nc.dram_tensor("rw1r", [128, HID], BF16, kind="ExternalInput")
    rw1i_d = nc.dram_tensor("rw1i", [128, 128], BF16, kind="ExternalInput")
    rw2_d = nc.dram_tensor("rw2f", [128, 2, HID], BF16, kind="ExternalInput")
    rw3_d = nc.dram_tensor("rw3f", [128, 2, HID], BF16, kind="ExternalInput")
    rw4_d = nc.dram_tensor("rw4b", [128, 2, EFF_D], BF16, kind="ExternalInput")
    ow1o_d = nc.dram_tensor("ow1o", [128, HID], BF16, kind="ExternalInput")
    ow1x_d = nc.dram_tensor("ow1x", [128, HID], BF16, kind="ExternalInput")
    ow1e_d = nc.dram_tensor("ow1e", [128, HID], BF16, kind="ExternalInput")
    ow2_d = nc.dram_tensor("ow2f", [128, 2, HID], BF16, kind="ExternalInput")
    ow3_d = nc.dram_tensor("ow3f", [128, 2, OUT_D], BF16, kind="ExternalInput")

    rb1_d = nc.dram_tensor("rb1c", [128, 2], F32, kind="ExternalInput")
    rb2_d = nc.dram_tensor("rb2c", [128, 2], F32, kind="ExternalInput")
    rb3_d = nc.dram_tensor("rb3c", [128, 2], F32, kind="ExternalInput")
    ob1_d = nc.dram_tensor("ob1c", [128, 2], F32, kind="ExternalInput")
    ob2_d = nc.dram_tensor("ob2c", [128, 2], F32, kind="ExternalInput")
    ob3_d = nc.dram_tensor("ob3r", [128, OUT_D], F32, kind="ExternalInput")
    rb4bc_d = nc.dram_tensor("rb4bc", [128, 512], BF16, kind="ExternalInput")

    out_d = nc.dram_tensor("out", [B_CORE, N_OBJ, OUT_D], F32, kind="ExternalOutput")

    chs = _chunks()
    LAST_CI = len(chs) - 1
    slots = [(b, ci) for b in range(B_CORE) for ci in range(len(chs))]
    NS = len(slots)

    with tile.TileContext(nc) as tc:
        with (
            tc.tile_pool(name="wts", bufs=1) as wts,
            tc.tile_pool(name="perb", bufs=1) as perb,
            tc.tile_pool(name="cin", bufs=5) as cin,
            tc.tile_pool(name="acts", bufs=3) as acts,
            tc.tile_pool(name="psp", bufs=5, space="PSUM") as psp,
            tc.tile_pool(name="ps4", bufs=2, space="PSUM") as ps4p,
            tc.tile_pool(name="psa", bufs=1, space="PSUM") as psap,
        ):
            # ---- small-matmul FIFO: one small per N=512 host matmul ----
            pend = deque()

            def pump():
                if pend:
                    pend.popleft()()

            def hostmm(*a, **k):
                nc.tensor.matmul(*a, skip_group_check=True, **k)
                pump()

            # ---- warm-up: PE busy early so the HAM clock gate goes 8/8
            # while the first DMAs land ----
            warm = wts.tile([128, 512], BF16)
            nc.gpsimd.memset(warm, 0.0)
            psW = psp.tile([128, 512], F32, tag="ps")
            for _ in range(N_WARM):
                nc.tensor.matmul(psW, warm[:, :128], warm, start=True, stop=True)

            # ---- setup DMAs.  sync: what the very first hosts need, then
            # the per-chunk streams (emitted in the slot loop).  scalar:
            # second-wave setup.  gpsimd: bulk weights. ----
            rw1s = wts.tile([128, HID], BF16)
            rw1r = wts.tile([128, HID], BF16)
            rw1i = wts.tile([128, 128], BF16)
            rb1 = wts.tile([128, 2], F32)
            OTs = [perb.tile([OBJ_D, N_OBJ], BF16, tag=f"OT{b}", name=f"OT{b}")
                   for b in range(B_CORE)]
            XTs = [perb.tile([128, N_OBJ], BF16, tag=f"XT{b}", name=f"XT{b}")
                   for b in range(B_CORE)]
            nc.sync.dma_start(rw1s, rw1s_d[:])
            nc.sync.dma_start(OTs[0], OT_d[0])
            nc.scalar.dma_start(rw1r, rw1r_d[:])
            nc.scalar.dma_start(rw1i, rw1i_d[:])
            # rb1 is read by L1 evacs from slot 0 on — its DMA must be
            # emitted before those reads or Tile orders it as a WAR
            nc.scalar.dma_start(rb1, rb1_d[:])

            # I-chunks for the ramp ride SWDGE ahead of the bulk weights so
            # the scalar (ACT) queue stays free for L1 evacuations
            I_ramp = []
            for ci in range(3):
                base, mc = chs[ci]
                I_c = cin.tile([128, M_CHUNK], BF16, tag="I_c", name=f"I_r{ci}")
                nc.gpsimd.dma_start(I_c[:, :mc], IT_d[0, :, base:base + mc])
                I_ramp.append(I_c)

            rw2 = wts.tile([128, 2, HID], BF16)
            rw3 = wts.tile([128, 2, HID], BF16)
            rw4 = wts.tile([128, 2, EFF_D], BF16)
            rb4bc = wts.tile([128, 512], BF16)
            ow1o = wts.tile([128, HID], BF16)
            ow1x = wts.tile([128, HID], BF16)
            ow1e = wts.tile([128, HID], BF16)
            ow2 = wts.tile([128, 2, HID], BF16)
            ow3 = wts.tile([128, 2, OUT_D], BF16)
            rb2 = wts.tile([128, 2], F32)
            rb3 = wts.tile([128, 2], F32)
            ob1 = wts.tile([128, 2], F32)
            ob2 = wts.tile([128, 2], F32)
            ob3 = wts.tile([128, OUT_D], F32)
            for t, dsrc in [(rw2, rw2_d), (rb2, rb2_d), (rw3, rw3_d),
                            (rb3, rb3_d), (rw4, rw4_d), (rb4bc, rb4bc_d)]:
                nc.gpsimd.dma_start(t, dsrc[:])
            obj_wt_dmas = [(ow1o, ow1o_d), (ow1x, ow1x_d), (ow1e, ow1e_d),
                           (ow2, ow2_d), (ow3, ow3_d),
                           (ob1, ob1_d), (ob2, ob2_d), (ob3, ob3_d)]

            # ---- per-slot / per-batch state ----
            ST, H1s, H2s, H3s, E3s = {}, {}, {}, {}, {}
            AsAr, psagg, aggfirst = {}, {}, {}

            def emit_AsAr(b):
                As = perb.tile([N_OBJ, HID], BF16, tag=f"As{b}")
                Ar = perb.tile([N_OBJ, HID], BF16, tag=f"Ar{b}")
                psA = psp.tile([128, 512], F32, tag="ps")
                hostmm(psA[:, :HID], OTs[b], rw1s, start=True, stop=True)
                nc.vector.tensor_copy(As, psA[:, :HID])
                psA2 = psp.tile([128, 512], F32, tag="ps")
                hostmm(psA2[:, :HID], OTs[b], rw1r, start=True, stop=True)
                nc.vector.tensor_copy(Ar, psA2[:, :HID])
                AsAr[b] = (As, Ar)

            def chunk_dmas(s):
                b, ci = slots[s]
                base, mc = chs[ci]
                S_c = cin.tile([N_OBJ, M_CHUNK], BF16, tag="S_c")
                R_c = cin.tile([N_OBJ, M_CHUNK], BF16, tag="R_c")
                I_c = (I_ramp[s] if s < 3 else
                       cin.tile([128, M_CHUNK], BF16, tag="I_c"))
                RT_c = cin.tile([128, M_CHUNK // 128, N_OBJ], BF16, tag="RT_c")
                ns = mc // 128
                rt_src = RT_d[b, base:base + mc, :].rearrange(
                    "(s p) o -> p s o", p=128)
                if s < 3:
                    # ramp: S/R/RT on sync, I pre-issued on SWDGE (I_ramp) so
                    # neither the sync trigger rate nor the scalar (ACT)
                    # queue starves the PE and re-throttles HAM
                    nc.sync.dma_start(S_c[:, :mc], S_d[b, :, base:base + mc])
                    nc.sync.dma_start(R_c[:, :mc], R_d[b, :, base:base + mc])
                    nc.sync.dma_start(RT_c[:, :ns, :], rt_src)
                else:
                    nc.sync.dma_start(S_c[:, :mc], S_d[b, :, base:base + mc])
                    nc.sync.dma_start(R_c[:, :mc], R_d[b, :, base:base + mc])
                    nc.sync.dma_start(I_c[:, :mc], IT_d[b, :, base:base + mc])
                    nc.sync.dma_start(RT_c[:, :ns, :], rt_src)
                ST[s] = (S_c, R_c, I_c, RT_c)

            def emit_L1(s):
                b, ci = slots[s]
                base, mc = chs[ci]
                As, Ar = AsAr[b]
                S_c, R_c, I_c, _ = ST[s]
                H1 = acts.tile([128, 2, M_CHUNK], BF16, tag="H1")
                for mt, n in _mtiles(mc):
                    sl = slice(mt, mt + n)
                    ps0 = psp.tile([128, 512], F32, tag="ps", name="ps0")
                    ps1 = psp.tile([128, 512], F32, tag="ps", name="ps1")
                    hostmm(ps0[:, :n], As[:, 0:128], S_c[:, sl], start=True, stop=False)
                    hostmm(ps0[:, :n], Ar[:, 0:128], R_c[:, sl], start=False, stop=False)
                    hostmm(ps1[:, :n], As[:, 128:256], S_c[:, sl], start=True, stop=False)
                    hostmm(ps1[:, :n], Ar[:, 128:256], R_c[:, sl], start=False, stop=False)
                    # adjacent K=64 row-tiled pair: rows 0:63 and 64:127 of
                    # the PE array compute both hid halves concurrently
                    nc.tensor.matmul(ps0[:, :n], rw1i[0:64, :], I_c[0:64, sl],
                                     start=False, stop=True, skip_group_check=True)
                    nc.tensor.matmul(ps1[:, :n], rw1i[64:128, :], I_c[64:128, sl],
                                     start=False, stop=True, skip_group_check=True)
                    pump()
                    nc.scalar.activation(H1[:, 0, sl], ps0[:, :n], RELU,
                                         bias=rb1[:, 0:1], scale=1.0)
                    nc.scalar.activation(H1[:, 1, sl], ps1[:, :n], RELU,
                                         bias=rb1[:, 1:2], scale=1.0)
                H1s[s] = H1

            def emit_L2(s):
                b, ci = slots[s]
                base, mc = chs[ci]
                H1 = H1s.pop(s)
                H2 = acts.tile([128, 2, M_CHUNK], BF16, tag="H2")
                for p2 in range(2):
                    h = slice(p2 * 128, (p2 + 1) * 128)
                    for mt, n in _mtiles(mc):
                        sl = slice(mt, mt + n)
                        ps = psp.tile([128, 512], F32, tag="ps")
                        hostmm(ps[:, :n], rw2[:, 0, h], H1[:, 0, sl], start=True, stop=False)
                        hostmm(ps[:, :n], rw2[:, 1, h], H1[:, 1, sl], start=False, stop=True)
                        nc.scalar.activation(H2[:, p2, sl], ps[:, :n], RELU,
                                             bias=rb2[:, p2:p2 + 1], scale=1.0)
                H2s[s] = H2

            def emit_L3(s):
                b, ci = slots[s]
                base, mc = chs[ci]
                H2 = H2s.pop(s)
                H3 = acts.tile([128, 2, M_CHUNK], BF16, tag="H3")
                for p2 in range(2):
                    h = slice(p2 * 128, (p2 + 1) * 128)
                    for mt, n in _mtiles(mc):
                        sl = slice(mt, mt + n)
                        ps = psp.tile([128, 512], F32, tag="ps")
                        hostmm(ps[:, :n], rw3[:, 0, h], H2[:, 0, sl], start=True, stop=False)
                        hostmm(ps[:, :n], rw3[:, 1, h], H2[:, 1, sl], start=False, stop=True)
                        if ci >= 16:
                            # small tail chunks: ACT is underloaded, DVE is
                            # the drain bottleneck
                            nc.scalar.activation(H3[:, p2, sl], ps[:, :n], RELU,
                                                 bias=rb3[:, p2:p2 + 1], scale=1.0)
                        else:
                            nc.vector.tensor_scalar(H3[:, p2, sl], ps[:, :n],
                                                    rb3[:, p2:p2 + 1], 0.0, ADD, MAX)
                H3s[s] = H3

            def enqueue_L4(s):
                b, ci = slots[s]
                base, mc = chs[ci]
                H3 = H3s.pop(s)
                ns = mc // 128
                E3 = acts.tile([128, M_CHUNK], BF16, tag="E3")
                E3s[s] = E3
                for g in range(0, ns, 4):
                    ge = min(g + 4, ns)
                    span = (ge - g) * 128
                    gsl = slice(g * 128, g * 128 + span)
                    ps4 = ps4p.tile([128, 512], F32, tag="ps4")
                    Etmp = acts.tile([128, 512], BF16, tag="Etmp")
                    for k in range(2):
                        for sj in range(g, ge):
                            last = (k == 1 and sj == ge - 1)

                            def t(k=k, sj=sj, g=g, ge=ge, ps4=ps4, Etmp=Etmp,
                                  span=span, gsl=gsl, last=last, H3=H3, E3=E3):
                                sl = slice(sj * 128, (sj + 1) * 128)
                                psl = slice((sj - g) * 128, (sj - g + 1) * 128)
                                nc.tensor.matmul(ps4[:, psl], H3[:, k, sl], rw4[:, k, :],
                                                 start=(k == 0 and sj == g),
                                                 stop=last,
                                                 skip_group_check=True)
                                if last:
                                    nc.vector.tensor_tensor(Etmp[:, :span], ps4[:, :span],
                                                            rb4bc[:, :span], ADD)
                                    if ci >= 16:
                                        nc.scalar.activation(E3[:, gsl], Etmp[:, :span],
                                                             RELU, bias=0.0, scale=1.0)
                                    else:
                                        nc.vector.tensor_scalar(E3[:, gsl], Etmp[:, :span],
                                                                0.0, None, MAX)
                            pend.append(t)

            def enqueue_agg(s):
                b, ci = slots[s]
                base, mc = chs[ci]
                ns = mc // 128
                E3 = E3s.pop(s)
                RT_c = ST.pop(s)[3]
                if b not in psagg:
                    psagg[b] = psap.tile([128, 512], F32, tag="agg",
                                         name=f"psagg{b}")
                    aggfirst[b] = [True]
                pa, first = psagg[b], aggfirst[b]
                last_chunk = (ci == LAST_CI)
                for sj in range(ns):
                    def t(sj=sj, pa=pa, first=first, E3=E3, RT_c=RT_c,
                          stop=(last_chunk and sj == ns - 1)):
                        nc.tensor.matmul(pa[:, :N_OBJ], E3[:, sj * 128:(sj + 1) * 128],
                                         RT_c[:, sj, :], start=first[0], stop=stop,
                                         skip_group_check=True)
                        first[0] = False
                    pend.append(t)

            def enqueue_obj(b):
                pa = psagg[b]
                effT = perb.tile([EFF_D, N_OBJ], BF16, tag=f"effT{b}")
                G1 = perb.tile([128, 2, N_OBJ], BF16, tag=f"G1{b}")
                G2 = perb.tile([128, 2, N_OBJ], BF16, tag=f"G2{b}")
                ob = perb.tile([N_OBJ, OUT_D], F32, tag=f"ob{b}")
                pend.append(lambda: nc.vector.tensor_copy(effT, pa[:, :N_OBJ]))
                for p2 in range(2):
                    ps = psp.tile([128, 512], F32, tag="ps")
                    h = slice(p2 * 128, (p2 + 1) * 128)
                    for wi, (wt, mv) in enumerate([(ow1o, OTs[b]), (ow1x, XTs[b]),
                                                   (ow1e, effT)]):
                        def t(ps=ps, h=h, wt=wt, mv=mv, wi=wi, p2=p2):
                            nc.tensor.matmul(ps[:, :N_OBJ], wt[:, h], mv,
                                             start=(wi == 0), stop=(wi == 2),
                                             skip_group_check=True)
                            if wi == 2:
                                # split evacs across ACT/DVE so the two
                                # halves retire in parallel on the tail path
                                if p2 == 0:
                                    nc.scalar.activation(G1[:, p2, :], ps[:, :N_OBJ],
                                                         RELU, bias=ob1[:, p2:p2 + 1],
                                                         scale=1.0)
                                else:
                                    nc.vector.tensor_scalar(G1[:, p2, :], ps[:, :N_OBJ],
                                                            ob1[:, p2:p2 + 1], 0.0,
                                                            ADD, MAX)
                        pend.append(t)
                for p2 in range(2):
                    ps = psp.tile([128, 512], F32, tag="ps")
                    h = slice(p2 * 128, (p2 + 1) * 128)
                    for k in range(2):
                        def t(ps=ps, h=h, k=k, p2=p2):
                            nc.tensor.matmul(ps[:, :N_OBJ], ow2[:, k, h], G1[:, k, :],
                                             start=(k == 0), stop=(k == 1),
                                             skip_group_check=True)
                            if k == 1:
                                if p2 == 0:
                                    nc.scalar.activation(G2[:, p2, :], ps[:, :N_OBJ],
                                                         RELU, bias=ob2[:, p2:p2 + 1],
                                                         scale=1.0)
                                else:
                                    nc.vector.tensor_scalar(G2[:, p2, :], ps[:, :N_OBJ],
                                                            ob2[:, p2:p2 + 1], 0.0,
                                                            ADD, MAX)
                        pend.append(t)
                pso = psp.tile([128, 512], F32, tag="ps")
                for k in range(2):
                    def t(pso=pso, k=k, b=b, ob=ob):
                        nc.tensor.matmul(pso[:, :OUT_D], G2[:, k, :], ow3[:, k, :],
                                         start=(k == 0), stop=(k == 1),
                                         skip_group_check=True)
                        if k == 1:
                            nc.vector.tensor_tensor(ob, pso[:, :OUT_D], ob3, ADD)
                            if b == 0:
                                nc.gpsimd.dma_start(out_d[b], ob)
                            else:
                                nc.sync.dma_start(out_d[b], ob)
                    pend.append(t)

            # ---- the continuous pipeline over both batches ----
            emit_AsAr(0)
            for s in range(NS):
                chunk_dmas(s)
                if s == 2 and obj_wt_dmas:
                    for t, dsrc in obj_wt_dmas:
                        nc.gpsimd.dma_start(t, dsrc[:])
                    obj_wt_dmas.clear()
                if s == 3:
                    nc.gpsimd.dma_start(OTs[1], OT_d[1])
                    nc.gpsimd.dma_start(XTs[0], XT_d[0])
                    nc.gpsimd.dma_start(XTs[1], XT_d[1])
                if s == 16:
                    emit_AsAr(1)
                if s >= 2:
                    q = s - 2
                    enqueue_agg(q)
                    if slots[q][1] == LAST_CI:
                        enqueue_obj(slots[q][0])
                emit_L1(s)
                if s >= 1:
                    emit_L3(s - 1)
                    enqueue_L4(s - 1)
                emit_L2(s)
            # drain
            emit_L3(NS - 1)
            enqueue_L4(NS - 1)
            enqueue_agg(NS - 2)
            enqueue_agg(NS - 1)
            enqueue_obj(slots[NS - 1][0])
            while pend:
                pump()

    nc.compile()
    return nc


def _prep_inputs(objects, sender_relations, receiver_relations, relation_info,
                 external_effect_info, rw1, rb1, rw2, rb2, rw3, rb3, rw4, rb4,
                 ow1, ob1, ow2, ob2, ow3, ob3):
    bf16 = ml_dtypes.bfloat16
    f32 = np.float32

    def a(x):
        return np.ascontiguousarray(np.asarray(x, dtype=f32))

    objects = a(objects); sender_relations = a(sender_relations)
    receiver_relations = a(receiver_relations); relation_info = a(relation_info)
    external_effect_info = a(external_effect_info)
    rw1, rb1, rw2, rb2, rw3, rb3, rw4, rb4 = map(a, (rw1, rb1, rw2, rb2, rw3, rb3, rw4, rb4))
    ow1, ob1, ow2, ob2, ow3, ob3 = map(a, (ow1, ob1, ow2, ob2, ow3, ob3))

    # relation info, transposed; rows 64:128 replicate rows 0:64 so the two
    # PE-row-tiled K=64 info matmuls can stream from partitions 0:64 / 64:128
    info_t = np.zeros((B, 128, N_REL), dtype=bf16)
    info_t[:, :REL_D, :] = relation_info.transpose(0, 2, 1).astype(bf16)
    info_t[:, REL_D:2 * REL_D, :] = info_t[:, :REL_D, :]
    s_bf = sender_relations.astype(bf16)
    r_bf = receiver_relations.astype(bf16)
    r_rel_t = np.ascontiguousarray(
        receiver_relations.transpose(0, 2, 1)).astype(bf16)
    objs_t = np.ascontiguousarray(objects.transpose(0, 2, 1)).astype(bf16)
    # ext, transposed and K-padded 32 -> 128
    ext_t = np.zeros((B, 128, N_OBJ), dtype=bf16)
    ext_t[:, :EXT_D, :] = external_effect_info.transpose(0, 2, 1).astype(bf16)

    # rw1i packed for row-tiling: rows 0:64 = info weights for hid half 0,
    # rows 64:128 = info weights for hid half 1
    rw1i_pad = np.zeros((128, 128), dtype=bf16)
    rw1i_pad[:REL_D] = rw1[256:320, 0:128].astype(bf16)
    rw1i_pad[64:64 + REL_D] = rw1[256:320, 128:256].astype(bf16)
    ow1x_pad = np.zeros((128, HID), dtype=bf16)
    ow1x_pad[:EXT_D] = ow1[128:160].astype(bf16)
    rb4bc = np.ascontiguousarray(
        np.broadcast_to(np.tile(rb4, 4).astype(bf16)[None, :], (128, 512)))

    shared = {
        "rw1s": rw1[0:128].astype(bf16),
        "rw1r": rw1[128:256].astype(bf16),
        "rw1i": rw1i_pad,
        "rw2f": np.ascontiguousarray(rw2.reshape(2, 128, HID).transpose(1, 0, 2)).astype(bf16),
        "rw3f": np.ascontiguousarray(rw3.reshape(2, 128, HID).transpose(1, 0, 2)).astype(bf16),
        "rw4b": np.ascontiguousarray(rw4.reshape(2, 128, EFF_D).transpose(1, 0, 2)).astype(bf16),
        "ow1o": ow1[0:128].astype(bf16),
        "ow1x": ow1x_pad,
        "ow1e": ow1[160:288].astype(bf16),
        "ow2f": np.ascontiguousarray(ow2.reshape(2, 128, HID).transpose(1, 0, 2)).astype(bf16),
        "ow3f": np.ascontiguousarray(ow3.reshape(2, 128, OUT_D).transpose(1, 0, 2)).astype(bf16),
        "rb1c": np.ascontiguousarray(rb1.reshape(2, 128).T),
        "rb2c": np.ascontiguousarray(rb2.reshape(2, 128).T),
        "rb3c": np.ascontiguousarray(rb3.reshape(2, 128).T),
        "ob1c": np.ascontiguousarray(ob1.reshape(2, 128).T),
        "ob2c": np.ascontiguousarray(ob2.reshape(2, 128).T),
        "ob3r": np.ascontiguousarray(np.broadcast_to(ob3[None, :], (128, OUT_D))),
        "rb4bc": rb4bc,
    }

    in_maps = []
    for c in range(N_CORES):
        sl = slice(c * B_CORE, (c + 1) * B_CORE)
        m = dict(shared)
        m["s_rel"] = s_bf[sl]
        m["r_rel"] = r_bf[sl]
        m["info_t"] = np.ascontiguousarray(info_t[sl])
        m["r_rel_t"] = r_rel_t[sl]
        m["objs_t"] = objs_t[sl]
        m["ext_t"] = np.ascontiguousarray(ext_t[sl])
        in_maps.append(m)
    return in_maps


def run(in_maps, **spmd_kwargs):
    from concourse.bass_utils import run_bass_kernel_spmd

    if "nc" not in _CACHE:
        _CACHE["nc"] = build_kernel()
    return run_bass_kernel_spmd(_CACHE["nc"], in_maps,
                                core_ids=list(range(N_CORES)), **spmd_kwargs)


def kernel(**inputs) -> np.ndarray:
    in_maps = _prep_inputs(**inputs)
    res = run(in_maps)
    out = np.concatenate([r["out"].reshape(-1, OUT_D) for r in res.results], axis=0)
    return np.ascontiguousarray(out, dtype=np.float32)
